# revision 14
# baseline (speedup 1.0000x reference)
"""DirectedGraphConvolution Trainium2 kernel (bf16/fp8 pipeline).

Per batch element b (one per NeuronCore, 8 total, data-parallel):
    N_e = H @ W                          [n, dout]
    T1  = G  @ N_e ; rs = G @ 1          (B1, during G arrival)
    T2  = G.T @ N_e ; cs = G.T @ 1       (A, post-residency)
    out = [ relu(0.5*(T1 + T2)),
            relu(G.T @ (T1 / rs)),       (C, fp8 DoubleRow)
            relu(G  @ (T2 / cs)) ]       (B2, fp8 DoubleRow)

Schedule: the 16 G row-tiles stream in f32 (~42us wire time).  As each
tile lands it is cast to bf16 (DVE), transposed on the PE (bf16, to
PSUM), and the transposed blocks are copied out twice: ACT -> a 4-deep
bf16 GT ring (B1 stationary), Pool -> a persistent fp8 GT image (B2
stationary).  B1 (T1 = G@[1 1|Ne]) consumes the ring immediately: its
contraction runs over columns, so each row-tile finishes in a single
PSUM bank -- the whole product hides under the DMA window, and rs
falls out of two ones-columns (no DVE reduction).  Post-residency:
pass A (T2/cs, jt-outer, bf16) with out1 fused into its epilogue,
then B2 and C as fp8 DoubleRow matmuls (2 k-tiles per instruction)
with scale factors 64/1024 folded into the cast + relu epilogues.
All matmul accumulation stays f32 in PSUM; only operand storage is
bf16/fp8 (validated ~3e-3 scale-relative error vs the 2e-2 gate).
"""

import numpy as np
import concourse.bass as bass
import concourse.mybir as mybir
import concourse.tile as tile
from concourse import bacc
from concourse.bass_utils import run_bass_kernel_spmd
from concourse.masks import make_identity

F32 = mybir.dt.float32
F32R = mybir.dt.float32r
BF16 = mybir.dt.bfloat16
FP8 = mybir.dt.float8e4
RELU = mybir.ActivationFunctionType.Relu
COPY = mybir.ActivationFunctionType.Copy
DR = mybir.MatmulPerfMode.DoubleRow

P = 128
B = 8
N = 2048
NO = N // P            # 16 row tiles
DIN = 256
DOUT = 256
KO = DIN // P          # 2 k tiles for H @ W
W3 = 3 * DOUT
RA = 2 + DOUT          # rhsb_a columns: [ones ones | N_e]
SC_T2 = 64.0           # fp8 scale for T2' = T2/cs
SC_T1 = 1024.0         # fp8 scale for T1' = T1/rs
GRING = 3              # GT bf16 ring depth (PE is in-order: T(it+k) issues after B1(it))
GSTG = 3               # f32 staging ring depth for arriving G tiles


def build():
    nc = bacc.Bacc("TRN2", target_bir_lowering=False)
    G = nc.declare_dram_parameter("G", [N, N], F32, isOutput=False)
    H = nc.declare_dram_parameter("H", [N, DIN], F32, isOutput=False)
    W = nc.declare_dram_parameter("W", [DIN, DOUT], F32, isOutput=False)
    out = nc.declare_dram_parameter("out", [N, W3], F32, isOutput=True)

    G_r = G.rearrange("(o p) j -> p o j", p=P)
    H_r = H.rearrange("(o p) d -> p o d", p=P).bitcast(F32R)
    W_r = W.rearrange("(o p) d -> p o d", p=P).bitcast(F32R)
    out_r = out.rearrange("(o p) d -> p o d", p=P)

    with tile.TileContext(nc) as tc:
        with (
            tc.tile_pool(name="const", bufs=1) as const,
            tc.tile_pool(name="gstg", bufs=GSTG) as gstg,
            tc.tile_pool(name="hstg", bufs=4) as hstg,
            tc.tile_pool(name="big", bufs=1) as big,
            tc.tile_pool(name="ring", bufs=GRING) as ringp,
            tc.tile_pool(name="hin", bufs=3) as hin,
            tc.tile_pool(name="stage", bufs=2) as stage,
            tc.tile_pool(name="tmpp", bufs=2) as tmpp,
        ):
            # ---- persistent SBUF images ----
            g_bf = big.tile([P, NO, N], BF16, name="g_bf")       # G bf16 (A stationary)
            gt_f8 = big.tile([P, NO, N], FP8, name="gt_f8")      # G^T fp8 [k-in-jt, jt, i] (B2)
            g_f8 = big.tile([P, NO, N], FP8, name="g_f8")        # G fp8 (C stationary)
            rhsa = big.tile([P, NO, RA], BF16, name="rhsa")      # [1 1 | N_e] per tile
            t2p = big.tile([P, NO, DOUT], FP8, name="t2p")       # T2' * 64  (B2 moving)
            t1p = big.tile([P, NO, DOUT], FP8, name="t1p")       # T1' * 1024 (C moving)
            t1b = big.tile([P, NO, DOUT], BF16, name="t1b")      # T1 raw (out1)
            rsinv = const.tile([P, NO, 1], F32)
            csinv = const.tile([P, NO, 1], F32)

            # staging for arriving G tiles (f32)
            g_stg = [gstg.tile([P, N], F32, tag="gs", name=f"gs{o}") for o in range(NO)]
            # G DMAs own the sync queue; H/W go on the scalar queue.
            for o in range(NO):
                nc.sync.dma_start(g_stg[o][:, 0:N // 2], G_r[:, o, 0:N // 2])
                nc.sync.dma_start(g_stg[o][:, N // 2:N], G_r[:, o, N // 2:N])

            w_sb = const.tile([P, KO, DOUT], F32R)
            nc.scalar.dma_start(w_sb, W_r)
            h_stg = [hstg.tile([P, DIN], F32R, tag="hs", name=f"hs{t}") for t in range(NO)]
            for t in range(NO):
                nc.scalar.dma_start(h_stg[t], H_r[:, t, :])

            ident_f32 = const.tile([P, P], F32)
            make_identity(nc, ident_f32)
            ident_bf = const.tile([P, P], BF16)
            nc.vector.tensor_copy(ident_bf, ident_f32)
            ident_r = const.tile([P, P], F32R)
            nc.vector.tensor_copy(ident_r, ident_f32)
            nc.gpsimd.memset(rhsa[:, :, 0:2], 1.0)

            gt_ring = [
                ringp.tile([P, NO, P], BF16, tag="gt", name=f"gt{r}")
                for r in range(GRING)
            ]

            with (
                tc.tile_pool(name="ps_ht", bufs=2, space="PSUM") as ps_ht,
                tc.tile_pool(name="ps_ne", bufs=2, space="PSUM") as ps_ne,
                tc.tile_pool(name="psT", bufs=2, space="PSUM") as psT,
                tc.tile_pool(name="psB1", bufs=2, space="PSUM") as psB1,
            ):
                # ---- N_e = H @ W (f32r), cast into rhsa bf16 ----
                hts = {}
                for t in range(NO + 1):
                    if t < NO:
                        ht_t = hin.tile([P, KO, P], F32R, tag="ht")
                        for kt in range(KO):
                            pt = ps_ht.tile([P, P], F32, tag="pht")
                            nc.tensor.transpose(
                                pt.bitcast(F32R),
                                h_stg[t][:, kt * P:(kt + 1) * P],
                                ident_r,
                            )
                            nc.vector.tensor_copy(ht_t[:, kt, :], pt.bitcast(F32R))
                        hts[t] = ht_t
                    if t >= 1:
                        u = t - 1
                        ht_u = hts.pop(u)
                        pne = ps_ne.tile([P, DOUT], F32, tag="pne")
                        for kt in range(KO):
                            nc.tensor.matmul(
                                pne,
                                ht_u[:, kt, :],
                                w_sb[:, kt, :],
                                start=(kt == 0),
                                stop=(kt == KO - 1),
                            )
                        nc.scalar.copy(rhsa[:, u, 2:RA], pne)

                # ---- arrival loop: per G tile, cast + transpose + B1 ----
                for it in range(NO):
                    # cast f32 staging -> bf16 (DVE)
                    nc.vector.tensor_copy(g_bf[:, it, :], g_stg[it])
                    # 16 transposes in 4 psum groups of 4
                    ring = gt_ring[it % GRING]
                    for q in range(4):
                        pt4 = psT.tile([P, 4, P], BF16, tag="pt4")
                        for s in range(4):
                            jt = 4 * q + s
                            nc.tensor.transpose(
                                pt4[:, s, :],
                                g_bf[:, it, jt * P:(jt + 1) * P],
                                ident_bf,
                            )
                        # ACT: batch-copy 4 blocks into the bf16 ring
                        nc.scalar.copy(ring[:, 4 * q:4 * (q + 1), :], pt4)
                        # Pool: same 4 blocks SBUF->SBUF into the fp8 GT image
                        # (GPSIMD cannot read PSUM)
                        nc.gpsimd.tensor_copy(
                            gt_f8[:, 4 * q:4 * (q + 1), it * P:(it + 1) * P],
                            ring[:, 4 * q:4 * (q + 1), :],
                        )
                    # B1: [rs rs | T1] = G @ [1 1 | N_e] for this row tile
                    pb1 = psB1.tile([P, RA], F32, tag="pb1")
                    for jt in range(NO):
                        nc.tensor.matmul(
                            pb1,
                            ring[:, jt, :],
                            rhsa[:, jt, :],
                            start=(jt == 0),
                            stop=(jt == NO - 1),
                        )
                    # epilogue: 1024/rs, T1 bf16, T1' fp8 (*1024)
                    ri = tmpp.tile([P, 1], F32, tag="ri")
                    nc.vector.tensor_scalar_mul(ri, pb1[:, 0:1], 1.0 / SC_T1)
                    nc.vector.reciprocal(rsinv[:, it, :], ri)  # 1024/rs
                    nc.vector.tensor_copy(t1b[:, it, :], pb1[:, 2:RA])
                    nc.vector.tensor_scalar_mul(
                        t1p[:, it, :], pb1[:, 2:RA], rsinv[:, it, 0:1]
                    )

            # ---- pass A: [cs cs | T2] = G.T @ [1 1 | N_e], jt-outer ----
            with tc.tile_pool(name="psA", bufs=3, space="PSUM") as psA:
                for jt in range(NO):
                    pa = psA.tile([P, RA], F32, tag="pa")
                    for it in range(NO):
                        nc.tensor.matmul(
                            pa,
                            g_bf[:, it, jt * P:(jt + 1) * P],
                            rhsa[:, it, :],
                            start=(it == 0),
                            stop=(it == NO - 1),
                        )
                    # epilogue: cs, T2' fp8 (*64), out1 = relu(0.5*(T1+T2))
                    ci = tmpp.tile([P, 1], F32, tag="ci")
                    nc.vector.tensor_scalar_mul(ci, pa[:, 0:1], 1.0 / SC_T2)
                    nc.vector.reciprocal(csinv[:, jt, :], ci)  # 64/cs
                    nc.vector.tensor_scalar_mul(
                        t2p[:, jt, :], pa[:, 2:RA], csinv[:, jt, 0:1]
                    )
                    o1 = stage.tile([P, DOUT], F32, tag="o1")
                    nc.vector.tensor_add(o1, pa[:, 2:RA], t1b[:, jt, :])
                    o1r = stage.tile([P, DOUT], F32, tag="o1r")
                    nc.scalar.activation(o1r, o1, RELU, scale=0.5)
                    nc.sync.dma_start(out_r[:, jt, 0:DOUT], o1r)
                    # fp8 copies of G for pass C ride the idle engines here
                    if jt % 2 == 0:
                        nc.gpsimd.tensor_copy(g_f8[:, jt // 2, :], g_bf[:, jt // 2, :])
                    else:
                        o = 8 + jt // 2
                        nc.vector.tensor_copy(g_f8[:, o, :], g_bf[:, o, :])

            # ---- pass B2: out3 = relu(G @ T2' / 64), fp8 DoubleRow ----
            with tc.tile_pool(name="psB2", bufs=3, space="PSUM") as psB2:
                for it in range(NO):
                    pb2 = psB2.tile([P, DOUT], F32, tag="pb2")
                    for jp in range(NO // 2):
                        nc.tensor.matmul(
                            pb2,
                            gt_f8[:, 2 * jp:2 * jp + 2, it * P:(it + 1) * P],
                            t2p[:, 2 * jp:2 * jp + 2, :],
                            start=(jp == 0),
                            stop=(jp == NO // 2 - 1),
                            perf_mode=DR,
                        )
                    o3 = stage.tile([P, DOUT], F32, tag="o3")
                    nc.scalar.activation(o3, pb2, RELU, scale=1.0 / SC_T2)
                    nc.sync.dma_start(out_r[:, it, 2 * DOUT:W3], o3)

            # ---- pass C: out2 = relu(G.T @ T1' / 1024), fp8 DoubleRow ----
            with tc.tile_pool(name="psC", bufs=3, space="PSUM") as psC:
                for jt in range(NO):
                    pc = psC.tile([P, DOUT], F32, tag="pc")
                    for ip in range(NO // 2):
                        nc.tensor.matmul(
                            pc,
                            g_f8[:, 2 * ip:2 * ip + 2, jt * P:(jt + 1) * P],
                            t1p[:, 2 * ip:2 * ip + 2, :],
                            start=(ip == 0),
                            stop=(ip == NO // 2 - 1),
                            perf_mode=DR,
                        )
                    o2 = stage.tile([P, DOUT], F32, tag="o2")
                    nc.scalar.activation(o2, pc, RELU, scale=1.0 / SC_T1)
                    nc.sync.dma_start(out_r[:, jt, DOUT:2 * DOUT], o2)

    nc.compile()
    return nc


_NC = None


def _get_nc():
    global _NC
    if _NC is None:
        _NC = build()
    return _NC


def run(inputs: dict, trace: bool = False):
    """Run on 8 cores; returns (stacked_out [B,N,W3], BassKernelResults)."""
    H, G, W = inputs["H"], inputs["G"], inputs["W"]
    H = np.ascontiguousarray(H, dtype=np.float32)
    G = np.ascontiguousarray(G, dtype=np.float32)
    W = np.ascontiguousarray(W, dtype=np.float32)
    in_maps = [
        {"G": np.ascontiguousarray(G[b]), "H": np.ascontiguousarray(H[b]), "W": W}
        for b in range(B)
    ]
    nc = _get_nc()
    res = run_bass_kernel_spmd(nc, in_maps, core_ids=list(range(B)), trace=trace)
    out = np.stack([res.results[b]["out"] for b in range(B)], axis=0)
    return out, res


def kernel(H, G, W):
    out, _ = run({"H": H, "G": G, "W": W})
    return out
